# revision 27
# baseline (speedup 1.0000x reference)
"""Trainium2 Bass kernel for C = triu(triu(A) @ triu(B)), N=4096, fp32.

Math: with host-side triu masking of A and B, the product of upper-triangular
matrices is upper-triangular; for an output tile (m, n) (128x128 tile indices)
the contraction over k only gets contributions from k in [m, n].

Sharding (8 cores, SPMD, one NEFF): 2D grid, 4 row groups x 2 column groups.
Core j = (r = j%4, c = j//4):
  - rows:    core owns row-tiles m with m % 4 == r  (8 slots, 1024 rows)
  - columns: core owns col-tiles n with n % 2 == c  (16 tiles, 2048 cols),
             gathered into 4 local 512-wide supers u: n in {8u+c+2j, j=0..3}
This cuts per-core HBM traffic from ~28MB (replicated-B row sharding) to
~16MB: A is triangle-packed (4.7MB bf16), B is half-sharded with diagonal
trimming (~9.4MB), C is stored as bf16 (2.6MB).

All cores run the identical program. Where a core's actual triangle is
smaller than the program's loop bounds (k-start 4t vs true row 4t+r; column
group c=0 vs the c=1 loop shapes) the host-packed operands hold zeros, so the
extra matmuls accumulate zeros and stay correct.

Schedule: supers descending (u=3 first). Within a super, ko runs V-shaped
(8..kmax then 7..0): the first k-tiles feed >=3 row slots each, so tensor
demand stays under the ~360GB/s DMA bus from the first matmul, and the
high-demand 1-2-slot k-tiles run last, when the stream is far ahead. B is
streamed on the Sync queue in need order; A on the Scalar queue (a joint
single-queue stream was tried and slowed the tensor engine ~15% - 16 DMA
engines hammering one SBUF tile region contend with PE reads). PSUM start
flags are correct because every slot's first matmul in this order is its
widest (w0 grows with ko; ko<=8 is full width). Two junk-matmul blocks warm
the PE p-state before the first real matmul (the PE reaches full clock only
after ~3us of continuous activity) and keep it busy through the C-store
drain window (the core clocks down ~3us after the PE idles, which would run
the end-of-NEFF barrier at half speed).
"""

import os
import sys

for _p in ("/opt/trn_rl_repo", "/root/.axon_site/_ro/trn_rl_repo"):
    if _p not in sys.path:
        sys.path.insert(0, _p)

import numpy as np

N = 4096
P = 128
KT = 32  # k tiles
GR = 4  # row groups
GC = 2  # col groups
NSLOT = 8  # row slots per core
NSUP = 4  # local 512-wide supers per core
SW = 512
NCORES = 8

NCT = sum(2 * u + 2 for u in range(NSUP))  # 20 C tiles per core


def _w0(ko, u):
    # leftmost nonzero column (c=1 core) of local super u at k-tile ko
    return 128 * max(0, min(3, (ko - 8 * u) // 2))


def _ko_order(u):
    """V-shaped ko order; u=0 runs slot t=1 then t=0 (see _compute_order)."""
    kmaxu = 8 * u + 7
    if u == 0:
        return list(range(kmaxu + 1))
    return list(range(8, kmaxu + 1)) + list(range(7, -1, -1))


def _compute_order(u):
    """Program-order list of (ko, t, w0, start, stop) for super u.

    Every slot's first matmul (start=True) is its widest (w0 is minimal
    over its program positions), as PSUM accumulation regions require.
    """
    kmaxu = 8 * u + 7
    nslots = 2 * u + 2
    out = []
    if u == 0:
        # t=1's short leg first, then all of t=0: t=1's drain overlaps
        # t=0's matmuls instead of serializing after the last one
        for ko in range(4, 8):
            out.append((ko, 1, _w0(ko, 0), ko == 4, ko == 7))
        for ko in range(8):
            out.append((ko, 0, _w0(ko, 0), ko == 0, ko == 7))
        return out
    for ko in _ko_order(u):
        w0 = _w0(ko, u)
        for t in range(min(ko // 4, nslots - 1) + 1):
            start = ko == max(4 * t, 8)
            stop = (ko == kmaxu) if t >= 2 else (ko == 4 * t)
            out.append((ko, t, w0, start, stop))
    return out


def _b_chunks(u):
    """(kc0, kc1, w0c) B-load chunks for super u, in need order."""
    chunks = []
    # 4-aligned chunks in _ko_order sequence
    seen = set()
    for ko in _ko_order(u):
        kc0 = 4 * (ko // 4)
        if kc0 not in seen:
            seen.add(kc0)
            w0c = 256 if kc0 == 8 * u + 4 else 0
            chunks.append((kc0, kc0 + 4, w0c))
    if u == NSUP - 1:
        # split the first chunk so the first real matmul starts sooner
        chunks = [(8, 9, 0), (9, 12, 0)] + chunks[1:]
    return chunks


# pair consecutive drains into one store: (u, t) -> (pair index, half)
PAIR_MAP = {}
NPAIRS_C = 0
for _u in range(NSUP - 1, -1, -1):
    _stops = (
        list(range(2, 2 * _u + 2)) + [1, 0] if _u > 0 else [1, 0]
    )  # slot stop order within the super
    for _i in range(0, len(_stops), 2):
        PAIR_MAP[(_u, _stops[_i])] = (NPAIRS_C, 0)
        PAIR_MAP[(_u, _stops[_i + 1])] = (NPAIRS_C, 1)
        NPAIRS_C += 1


# A pack: pairs (t, ko), enumerated in u=3's program (consumption) order so A
# streams in exactly as needed
A_POS = {}
for _ko, _t, _w, _s, _p_ in _compute_order(3):
    if (_t, _ko) not in A_POS:
        A_POS[(_t, _ko)] = len(A_POS)
NPAIR = len(A_POS)  # 144

# chunk boundaries for the A stream (in A_POS units)
A_EDGES = [0, 4, 16, 32, 56, 88, 120, NPAIR]


def _dma_schedule():
    """All A/B chunk issues with need times (in matmul columns), sorted."""
    items = []  # (need_cols, kind, payload)
    # consumption clock: columns, accumulated over program order
    cols = 0
    a_need = {}  # pos -> cols at first use
    b_need = {}  # (u, ko) -> cols at first use
    for u in range(NSUP - 1, -1, -1):
        for ko, t, w0, start, stop in _compute_order(u):
            pos = A_POS[(t, ko)]
            a_need.setdefault(pos, cols)
            b_need.setdefault((u, ko), cols)
            cols += SW - w0
    for c0, c1 in zip(A_EDGES, A_EDGES[1:]):
        items.append((a_need[c0], "a", (c0, c1)))
    for u in range(NSUP):
        for kc0, kc1, w0c in _b_chunks(u):
            need = min(b_need[(u, ko)] for ko in range(kc0, kc1))
            items.append((need, "b", (u, kc0, kc1, w0c)))
    items.sort(key=lambda x: x[0])
    return items


# C store dtype: bf16 halves store traffic; rel-err budget (2e-2) dominated
# by bf16 matmul inputs either way
C_DTYPE = os.environ.get("C_DTYPE", "bf16")

_cache = {}


def _build(c_dtype):
    import concourse.bacc as bacc
    import concourse.mybir as mybir
    import concourse.tile as tile

    D = mybir.dt.bfloat16
    DC = mybir.dt.bfloat16 if c_dtype == "bf16" else mybir.dt.float32

    nc = bacc.Bacc(None, target_bir_lowering=False)
    ATp = nc.dram_tensor("ATp", [P, NPAIR, P], D, kind="ExternalInput")
    Bp = nc.dram_tensor("B", [NSUP, P, KT, SW], D, kind="ExternalInput")
    Cp = nc.dram_tensor("C", [NPAIRS_C, P, 2, SW], DC, kind="ExternalOutput")

    with tile.TileContext(nc) as tc:
        with (
            tc.tile_pool(name="a", bufs=1) as apool,
            tc.tile_pool(name="b", bufs=3) as bpool,
            tc.tile_pool(name="o", bufs=4) as opool,
            tc.tile_pool(name="w", bufs=2) as wpool,
            tc.tile_pool(name="ps", bufs=8, space="PSUM") as pspool,
        ):
            a_res = apool.tile([P, NPAIR, P], D, tag="a", name="ar")
            bts = {
                u: bpool.tile([P, KT, SW], D, tag="b", name="bt")
                for u in range(NSUP - 1, -1, -1)
            }
            # A on the Scalar queue (needs ~154KB/us, under a half-bus
            # share), B alone on Sync in need order
            for _need, kind, payload in _dma_schedule():
                if kind == "a":
                    c0, c1 = payload
                    nc.scalar.dma_start(a_res[:, c0:c1, :], ATp[:, c0:c1, :])
                else:
                    u, kc0, kc1, w0c = payload
                    nc.sync.dma_start(
                        bts[u][:, kc0:kc1, w0c:], Bp[u, :, kc0:kc1, w0c:]
                    )

            # PE warmup: junk matmuls on the first A chunk while the first
            # B chunk is still in flight. The PE p-state ramps to full clock
            # only after ~3us of continuous execution; without this the
            # first ~25k columns run at ~1.5 cyc/col.
            warm_ps = pspool.tile([P, SW], mybir.dt.float32, tag="ps", name="wp")
            for w in range(16):
                nc.tensor.matmul(
                    warm_ps[:, :P],
                    a_res[:, 0, :],
                    a_res[:, w % 4, :],
                    start=(w == 0),
                    stop=(w == 15),
                )
            warm_ot = wpool.tile([P, SW], D, tag="w", name="wo")
            nc.vector.tensor_copy(warm_ot[:], warm_ps[:])

            pair_ots = {}
            for u in range(NSUP - 1, -1, -1):
                kmaxu = 8 * u + 7
                nslots = 2 * u + 2
                bt = bts[u]
                # allocate in descending t: the pool hands buffers in
                # allocation order, and descending maps this super's
                # latest-used slots onto the previous super's latest-freed
                # banks (and first-used onto earliest-freed) - no drain wait
                psums = {}
                for t in range(nslots - 1, -1, -1):
                    psums[t] = pspool.tile(
                        [P, SW], mybir.dt.float32, tag="ps", name="ps"
                    )

                def drain(t, u=u, psums=psums):
                    # two casts share one [P, 2, SW] tile and one store:
                    # halves the C DMA issues (and their semaphores)
                    q, h = PAIR_MAP[(u, t)]
                    if q not in pair_ots:
                        pair_ots[q] = opool.tile(
                            [P, 2, SW], DC, tag="o", name="ot"
                        )
                    nc.vector.tensor_copy(pair_ots[q][:, h, :], psums[t][:])
                    if h == 1:
                        # C stores ride the GpSimd queue, off the input stream
                        nc.gpsimd.dma_start(Cp[q], pair_ots[q][:])

                for ko, t, w0, start, stop in _compute_order(u):
                    nc.tensor.matmul(
                        psums[t][:, w0:],
                        a_res[:, A_POS[(t, ko)], :],
                        bt[:, ko, w0:],
                        start=start,
                        stop=stop,
                    )
                    if stop:
                        drain(t)

            # anti-idle: the core clocks down ~3us after the tensor engine
            # goes idle, which would run the end-of-NEFF barrier at half
            # speed; keep the PE busy through the C-store drain window
            warm2 = pspool.tile([P, SW], mybir.dt.float32, tag="ps", name="w2")
            for w in range(12):
                nc.tensor.matmul(
                    warm2[:],
                    a_res[:, 0, :],
                    warm_ot[:],
                    start=(w == 0),
                    stop=(w == 11),
                )
            warm2_ot = wpool.tile([P, SW], D, tag="w", name="w2o")
            nc.vector.tensor_copy(warm2_ot[:], warm2[:])
    nc.compile()
    return nc


def _get_nc():
    if C_DTYPE not in _cache:
        _cache[C_DTYPE] = _build(C_DTYPE)
    return _cache[C_DTYPE]


def _make_in_maps(A, B):
    import ml_dtypes

    bf16 = np.dtype(ml_dtypes.bfloat16)
    A32 = np.asarray(A, dtype=np.float32)
    B32 = np.asarray(B, dtype=np.float32)
    Au = np.triu(A32).astype(bf16)
    Bu = np.triu(B32).astype(bf16)

    # B pack per column group c:
    #   Bp[u, p, ko, 128*j + wc] = Bu[128*ko + p, 128*(8u + c + 2j) + wc]
    Xb = Bu.reshape(KT, P, KT, P)
    Bpacks = []
    for c in range(GC):
        Bp = np.stack(
            [
                np.ascontiguousarray(
                    Xb[:, :, 8 * u + c : 8 * u + c + 8 : 2, :]
                    .transpose(1, 0, 2, 3)
                    .reshape(P, KT, SW)
                )
                for u in range(NSUP)
            ]
        )
        Bpacks.append(Bp)

    # A pack per row group r: lhsT pairs (t, ko) in consumption order:
    #   ATp[p, A_POS[(t, ko)], ml] = Au[128*(4t+r) + ml, 128*ko + p]
    ATpacks = []
    for r in range(GR):
        ATp = np.empty((P, NPAIR, P), dtype=bf16)
        for (t, ko), pos in A_POS.items():
            m = 4 * t + r
            ATp[:, pos, :] = Au[P * m : P * (m + 1), P * ko : P * (ko + 1)].T
        ATpacks.append(ATp)

    in_maps = []
    for j in range(NCORES):
        r, c = j % GR, j // GR
        in_maps.append({"ATp": ATpacks[r], "B": Bpacks[c]})
    return in_maps


def kernel(A, B):
    from concourse.bass_utils import run_bass_kernel_spmd

    in_maps = _make_in_maps(A, B)
    nc = _get_nc()
    res = run_bass_kernel_spmd(nc, in_maps, core_ids=list(range(NCORES)))

    C = np.zeros((N, N), dtype=np.float32)
    for j in range(NCORES):
        r, c = j % GR, j // GR
        Cj = res.results[j]["C"]
        for u in range(NSUP):
            for t in range(2 * u + 2):
                m = 4 * t + r
                q, h = PAIR_MAP[(u, t)]
                tile_ = Cj[q][:, h, :]
                for jj in range(4):
                    n = 8 * u + c + 2 * jj
                    if n >= m:
                        C[P * m : P * (m + 1), P * n : P * (n + 1)] = tile_[
                            :, P * jj : P * (jj + 1)
                        ].astype(np.float32)
    return C
